# revision 1
# baseline (speedup 1.0000x reference)
"""DST Encoder (Augment -> depth-2 stream signature -> 2-layer GRU).

Data-parallel across 8 NeuronCores: batch B=64 is sharded 8 ways (8
samples per core); all conv/GRU parameters are replicated. Per-sample
cumsum over time and the GRU scans need no cross-device communication.
The full (B, L, C, C) level-2 signature tensor is only ever materialized
per-shard (8 samples -> 33 MB), keeping each core's working set small.

Hardcoded problem shapes: B=64, L=256, D_IN=55, C=64, H=64.
"""

import numpy as np
import jax
import jax.numpy as jnp

B, L, D_IN = 64, 256, 55
C = 64          # D_IN + 1 (time) + 8 (augmented)
H = 64          # GRU hidden
N_SHARDS = 8

_PARAM_NAMES = (
    "conv_w1", "conv_b1", "conv_w2", "conv_b2",
    "gru_Wih0", "gru_Whh0", "gru_bih0", "gru_bhh0",
    "gru_Wih1", "gru_Whh1", "gru_bih1", "gru_bhh1",
)


def _forward(x, conv_w1, conv_b1, conv_w2, conv_b2,
             gru_Wih0, gru_Whh0, gru_bih0, gru_bhh0,
             gru_Wih1, gru_Whh1, gru_bih1, gru_bhh1):
    # ---- Augment: pointwise conv stack, concat [x, time, aug] ----
    h = jax.nn.relu(jnp.einsum("bld,hd->blh", x, conv_w1) + conv_b1)
    aug = jnp.einsum("blh,ah->bla", h, conv_w2) + conv_b2
    t = jnp.linspace(0.0, 1.0, L, dtype=x.dtype)
    time = jnp.broadcast_to(t[None, :, None], (x.shape[0], L, 1))
    p = jnp.concatenate([x, time, aug], axis=-1)            # (b, L, C)

    # ---- Depth-2 streaming signature (zero basepoint) ----
    dx = p - jnp.concatenate([jnp.zeros_like(p[:, :1]), p[:, :-1]], axis=1)
    s1 = jnp.cumsum(dx, axis=1)                             # (b, L, C)
    s1_prev = s1 - dx
    # Level-2 increment is rank-1 per step: (s1_prev + dx/2) (x) dx.
    a = s1_prev + 0.5 * dx
    # Project the level-2 signature through Wih0 without materializing the
    # full (b, L, C*C) cumsum in fp32 HBM at full batch: cumsum commutes
    # with the (linear) input projection, so project per-step increments
    # then cumsum the (b, L, 3H) result.
    W1 = gru_Wih0[:, :C]                                    # (3H, C)
    W2 = gru_Wih0[:, C:].reshape(3 * H, C, C)               # (3H, C, C)
    z2 = jnp.einsum("bli,gij,blj->blg", a, W2, dx)          # (b, L, 3H)
    xg0 = jnp.einsum("blc,gc->blg", s1, W1) + jnp.cumsum(z2, axis=1) + gru_bih0

    # ---- GRU layers (PyTorch gate order r, z, n) ----
    def make_step(Whh, bhh):
        def step(h, g_t):
            gh = h @ Whh.T + bhh
            ir, iz, inn = jnp.split(g_t, 3, axis=-1)
            hr, hz, hn = jnp.split(gh, 3, axis=-1)
            r = jax.nn.sigmoid(ir + hr)
            z = jax.nn.sigmoid(iz + hz)
            n = jnp.tanh(inn + r * hn)
            h_new = (1.0 - z) * n + z * h
            return h_new, h_new
        return step

    h0 = jnp.zeros((x.shape[0], H), x.dtype)
    _, ys0 = jax.lax.scan(make_step(gru_Whh0, gru_bhh0), h0,
                          jnp.swapaxes(xg0, 0, 1))
    seq1 = jnp.swapaxes(ys0, 0, 1)                          # (b, L, H)

    xg1 = jnp.einsum("blc,gc->blg", seq1, gru_Wih1) + gru_bih1
    _, ys1 = jax.lax.scan(make_step(gru_Whh1, gru_bhh1), h0,
                          jnp.swapaxes(xg1, 0, 1))
    return jnp.swapaxes(ys1, 0, 1)                          # (b, L, H)


def _forward_np(x, conv_w1, conv_b1, conv_w2, conv_b2,
                gru_Wih0, gru_Whh0, gru_bih0, gru_bhh0,
                gru_Wih1, gru_Whh1, gru_bih1, gru_bhh1):
    h = np.maximum(x @ conv_w1.T + conv_b1, 0.0)
    aug = h @ conv_w2.T + conv_b2
    t = np.linspace(0.0, 1.0, L, dtype=np.float32)
    time = np.broadcast_to(t[None, :, None], (x.shape[0], L, 1))
    p = np.concatenate([x, time, aug], axis=-1)
    dx = p.copy()
    dx[:, 1:] -= p[:, :-1]
    s1 = np.cumsum(dx, axis=1, dtype=np.float32)
    a = (s1 - dx) + 0.5 * dx
    W1 = gru_Wih0[:, :C]
    W2r = gru_Wih0[:, C:].reshape(3 * H, C, C)
    # z2[b,l,g] = a[b,l,:] @ W2r[g] @ dx[b,l,:], batched as two matmuls.
    am = np.einsum("bli,gij->blgj", a, W2r, optimize=True)
    z2 = np.einsum("blgj,blj->blg", am, dx, optimize=True)
    xg0 = s1 @ W1.T + np.cumsum(z2, axis=1, dtype=np.float32) + gru_bih0

    def sig(v):
        return 1.0 / (1.0 + np.exp(-v))

    def run_gru(xg, Whh, bhh):
        b = xg.shape[0]
        hh = np.zeros((b, H), np.float32)
        ys = np.empty((b, L, H), np.float32)
        for ti in range(L):
            gh = hh @ Whh.T + bhh
            g_t = xg[:, ti]
            r = sig(g_t[:, :H] + gh[:, :H])
            z = sig(g_t[:, H:2 * H] + gh[:, H:2 * H])
            n = np.tanh(g_t[:, 2 * H:] + r * gh[:, 2 * H:])
            hh = (1.0 - z) * n + z * hh
            ys[:, ti] = hh
        return ys

    seq1 = run_gru(xg0.astype(np.float32), gru_Whh0, gru_bhh0)
    xg1 = seq1 @ gru_Wih1.T + gru_bih1
    return run_gru(xg1.astype(np.float32), gru_Whh1, gru_bhh1)


def kernel(**inputs: np.ndarray) -> np.ndarray:
    x = np.asarray(inputs["x"], dtype=np.float32)
    params = [np.asarray(inputs[n], dtype=np.float32) for n in _PARAM_NAMES]

    try:
        n_dev = len(jax.local_devices())
        if n_dev >= N_SHARDS:
            # Shard batch across 8 cores, replicate params.
            xs = x.reshape(N_SHARDS, B // N_SHARDS, L, D_IN)
            fn = jax.pmap(_forward, in_axes=(0,) + (None,) * len(params))
            out = fn(xs, *params)                           # (8, 8, L, H)
            out = np.asarray(out).reshape(B, L, H)
        else:
            out = np.asarray(jax.jit(_forward)(x, *params))
    except Exception:
        out = _forward_np(x, *params)
    return out.astype(np.float32)


if __name__ == "__main__":
    rng = np.random.default_rng(0)
    demo = {"x": rng.standard_normal((B, L, D_IN), dtype=np.float32)}
    demo["conv_w1"] = rng.standard_normal((32, D_IN), dtype=np.float32) * 0.1
    demo["conv_b1"] = np.zeros(32, np.float32)
    demo["conv_w2"] = rng.standard_normal((8, 32), dtype=np.float32) * 0.1
    demo["conv_b2"] = np.zeros(8, np.float32)
    for l, d in ((0, C + C * C), (1, H)):
        demo[f"gru_Wih{l}"] = rng.standard_normal((3 * H, d), dtype=np.float32) * 0.05
        demo[f"gru_Whh{l}"] = rng.standard_normal((3 * H, H), dtype=np.float32) * 0.05
        demo[f"gru_bih{l}"] = np.zeros(3 * H, np.float32)
        demo[f"gru_bhh{l}"] = np.zeros(3 * H, np.float32)
    print(kernel(**demo).shape)

